# revision 25
# baseline (speedup 1.0000x reference)
"""Fused binary-conv + sync-BN + hardtanh + sign kernel for Trainium2 (8 cores).

Reference computation (NCHW, fp32):
    y   = conv2d(x, sign(weight), 3x3, pad 1)
    y   = batchnorm(y, batch stats over (N,H,W), gamma, beta)
    out = sign(clip(y, -1, 1))         # == sign(y): clip never changes sign

Strategy:
  * Data-parallel: 32 images sharded 4-per-core across 8 NeuronCores.
  * Host pre-binarizes + transposes weights to [9, ci, co] (exact: +-1).
  * Conv as 9 shifted matmuls over a zero-padded [ci, 58, 58] SBUF image.
  * Default path: fp32r matmuls. Measured on this HW, fp32r keeps
    ~12.2 mantissa bits of the moving operand at bf16 throughput
    (1 cycle/row for free dim >= 256), and +-1 weights make the products
    exact — one accumulation pass instead of bf16 hi/lo's two. Measured
    853/25.7M sign flips (rel err 1.15e-2 < 2e-2 gate, deterministic
    inputs). KERNEL_VARIANT=bf16 restores the hi/lo split (27 flips,
    rel err 2.1e-3) at ~1.7x the tensor time.
  * PSUM tile = [co 128, 8 rows x 56 = 448]; 18 matmuls accumulate each
    (36 for the bf16 hi/lo path).
  * Epilogue per tile: ACT copy PSUM->SBUF with accum_out (= sum y),
    ACT Square with accum_out (= sum y^2). y stays resident in SBUF
    (100 KB/partition; KERNEL_VARIANT=ydram restores the DRAM stash —
    measured 208 us vs 232 us per exec on HW in favour of SBUF).
    (tensor_tensor_reduce would be the natural choice for the stats but
    hangs real HW through this runtime.)
  * One 2 KB AllReduce (8 cores) for global batch stats -> scale/shift.
  * Tail: sign(y*scale + shift) fused in a single ACT op per [128,3136]
    tile, SBUF->DRAM.

The builder supports `repeat=R`: the full workload (including weight/x
loads and the collective) is emitted R times into one NEFF, reusing the
same buffers. Output is idempotent across reps. test.py uses R>1 to
measure true HW execution time as the marginal dispatch-time per rep
(the fixed ~80-90 ms axon dispatch overhead cancels in the difference).
"""

import os
import sys
import numpy as np

for _p in ("/opt/trn_rl_repo",):
    if _p not in sys.path:
        sys.path.insert(0, _p)

import ml_dtypes  # noqa: E402
import concourse.bacc as bacc  # noqa: E402
import concourse.mybir as mybir  # noqa: E402
import concourse.tile as tile  # noqa: E402
from concourse import bass_utils  # noqa: E402

N_CORES = 8
NIMG = 4            # images per core
CI = CO = 256
H = W = 56
HP, WP = H + 2, W + 2
P = 128
RH = 8              # output rows per PSUM tile
NRB = H // RH       # 7 row blocks
NF = RH * W         # 448 free elems per PSUM tile
EPS = 1e-5
CNT = 32 * H * W    # BN count over full batch (N, H, W)

F32 = mybir.dt.float32
F32R = mybir.dt.float32r
BF16 = mybir.dt.bfloat16
AF = mybir.ActivationFunctionType
ALU = mybir.AluOpType

_CACHE = {}


def _build(n_cores=N_CORES, repeat=1):
    variant = os.environ.get("KERNEL_VARIANT", "")
    convonly = "convonly" in variant
    # keep conv output y resident in SBUF (100 KB/partition) instead of a
    # DRAM stash round-trip: measured 208 us vs 232 us per exec on HW
    ysbuf = "ydram" not in variant
    f32r = "bf16" not in variant

    nc = bacc.Bacc(
        "TRN2",
        target_bir_lowering=False,
        debug=False,
        enable_asserts=True,
        num_devices=n_cores,
    )
    x_t = nc.dram_tensor("x", [NIMG, CI, H, W], F32, kind="ExternalInput")
    # +-1 weights are exactly representable in every matmul dtype
    wdt = F32R if f32r else BF16
    wt_t = nc.dram_tensor("wt", [9, CI, CO], wdt, kind="ExternalInput")
    g_t = nc.dram_tensor("gamma", [CO], F32, kind="ExternalInput")
    b_t = nc.dram_tensor("beta", [CO], F32, kind="ExternalInput")
    o_t = nc.dram_tensor("out", [NIMG, CO, H, W], F32, kind="ExternalOutput")

    x_flat = x_t.ap().rearrange("n c h w -> n c (h w)")
    o_flat = o_t.ap().rearrange("n c h w -> n c (h w)")

    with tile.TileContext(nc) as tc:
        with tc.tile_pool(name="wpool", bufs=1) as wpool, \
             tc.tile_pool(name="data", bufs=1) as data, \
             tc.tile_pool(name="work", bufs=1) as work, \
             tc.tile_pool(name="pspool", bufs=1, space="PSUM") as pspool, \
             tc.tile_pool(name="drampool", bufs=1, space="DRAM") as drampool:

            # ---------- persistent tiles, shared across reps ----------
            w_lhsT = [[wpool.tile([P, CO], wdt, name=f"w_{pos}_{c}")
                       for c in range(2)] for pos in range(9)]
            gamma_sb = data.tile([P, 2], F32, name="gamma_sb")
            beta_sb = data.tile([P, 2], F32, name="beta_sb")
            sum_cols = [data.tile([P, NIMG * NRB], F32, name=f"sumc_{m}")
                        for m in range(2)]
            sq_cols = [data.tile([P, NIMG * NRB], F32, name=f"sqc_{m}")
                       for m in range(2)]

            y_dram = None
            y_sb = None
            if ysbuf:
                y_sb = [data.tile([P, H * W], F32, name=f"ysb_{i}")
                        for i in range(NIMG * 2)]
                xf_bufs = int(os.environ.get("K_XFBUFS", "2"))
                hl_bufs = int(os.environ.get("K_HLBUFS", "2"))
            else:
                y_dram = drampool.tile([NIMG * 2, P, H * W], F32,
                                       name="y_stash")
                xf_bufs = int(os.environ.get("K_XFBUFS", "3"))
                hl_bufs = int(os.environ.get("K_HLBUFS", "4"))

            stats_sb = data.tile([P, 4], F32, name="stats_sb")
            in_b = drampool.tile([P, 4], F32, name="ar_in")
            out_ads = "Shared" if "sharedcc" in variant else "Local"
            out_b = drampool.tile([P, 4], F32, name="ar_out",
                                  addr_space=out_ads)
            gstats = data.tile([P, 4], F32, name="gstats")
            mean_sb = data.tile([P, 2], F32, name="mean_sb")
            var_sb = data.tile([P, 2], F32, name="var_sb")
            msq = data.tile([P, 2], F32, name="msq")
            inv_sb = data.tile([P, 2], F32, name="inv_sb")
            rstd_sb = data.tile([P, 2], F32, name="rstd_sb")
            scale_sb = data.tile([P, 2], F32, name="scale_sb")
            mscale = data.tile([P, 2], F32, name="mscale")
            shift_sb = data.tile([P, 2], F32, name="shift_sb")
            if f32r:
                # zero sources for the f32r pad borders (no engine can
                # memset an f32r tile directly; DVE copy F32->F32R rounds)
                zrows = data.tile([P, 2, WP], F32, name="zrows")
                zcols = data.tile([P, H, 2], F32, name="zcols")

            for rep in range(repeat):
                # ---------- parameter loads ----------
                for pos in range(9):
                    for c in range(2):
                        nc.sync.dma_start(w_lhsT[pos][c][:],
                                          wt_t.ap()[pos, c * P:(c + 1) * P, :])
                nc.sync.dma_start(gamma_sb[:],
                                  g_t.ap().rearrange("(m p) -> p m", p=P))
                nc.sync.dma_start(beta_sb[:],
                                  b_t.ap().rearrange("(m p) -> p m", p=P))

                # ---------- conv + stats ----------
                if f32r:
                    nc.gpsimd.memset(zrows[:], 0.0)
                    nc.gpsimd.memset(zcols[:], 0.0)
                for img in range(NIMG):
                    srcs = []          # per ci-chunk: list of padded operands
                    for c in range(2):
                        x_f32 = work.tile([P, H * W], F32, tag="xf32",
                                          bufs=xf_bufs,
                                          name=f"x_{rep}_{img}_{c}")
                        nc.sync.dma_start(x_f32[:],
                                          x_flat[img, c * P:(c + 1) * P, :])
                        x3 = x_f32[:].rearrange("p (h w) -> p h w", h=H)
                        if f32r:
                            # single fp32r operand (~12.2 mantissa bits on
                            # HW) — measured flip rate fits the error budget.
                            # NOTE: DMA-ing x straight into the pad interior
                            # (dropping this staging tile) was tried and is
                            # WORSE on HW (231 us vs 173 us): the strided
                            # [128,56,56] destination breaks the transfer
                            # into 224 B/row descriptors vs one contiguous
                            # 12.5 KB row here. Contiguous DMA + DVE copy
                            # wins despite the extra SBUF traffic.
                            pad = work.tile([P, HP, WP], F32R, tag="hi",
                                            bufs=hl_bufs,
                                            name=f"pad_{rep}_{img}_{c}")
                            nc.vector.tensor_copy(
                                pad[:, 0:HP:HP - 1, :], zrows[:])
                            nc.vector.tensor_copy(
                                pad[:, 1:1 + H, 0:WP:WP - 1], zcols[:])
                            nc.vector.tensor_copy(pad[:, 1:1 + H, 1:1 + W],
                                                  x3)
                            srcs.append([pad])
                        else:
                            hi = work.tile([P, HP, WP], BF16, tag="hi",
                                           bufs=hl_bufs,
                                           name=f"hi_{rep}_{img}_{c}")
                            lo = work.tile([P, HP, WP], BF16, tag="lo",
                                           bufs=hl_bufs,
                                           name=f"lo_{rep}_{img}_{c}")
                            nc.gpsimd.memset(hi[:], 0.0)
                            nc.gpsimd.memset(lo[:], 0.0)
                            nc.vector.tensor_copy(hi[:, 1:1 + H, 1:1 + W], x3)
                            nc.vector.tensor_sub(lo[:, 1:1 + H, 1:1 + W], x3,
                                                 hi[:, 1:1 + H, 1:1 + W])
                            srcs.append([hi, lo])

                    n_mm = 9 * 2 * len(srcs[0])   # matmuls per PSUM tile
                    for m in range(2):
                        for rb in range(NRB):
                            ps = pspool.tile(
                                [P, NF], F32, tag="ps",
                                bufs=int(os.environ.get("K_PSBUFS", "8")),
                                name=f"ps_{rep}_{img}_{m}_{rb}")
                            n_acc = 0
                            for c in range(2):
                                for src in srcs[c]:
                                    for pos in range(9):
                                        kh, kw = divmod(pos, 3)
                                        rhs = src[:,
                                                  rb * RH + kh:
                                                  rb * RH + kh + RH,
                                                  kw: kw + W]
                                        nc.tensor.matmul(
                                            ps[:],
                                            w_lhsT[pos][c][:,
                                                           m * P:(m + 1) * P],
                                            rhs,
                                            start=(n_acc == 0),
                                            stop=(n_acc == n_mm - 1),
                                        )
                                        n_acc += 1
                            # epilogue: stage y, accumulate sum / sumsq
                            idx = img * NRB + rb
                            if y_sb is not None:
                                y_stage = y_sb[img * 2 + m][
                                    :, rb * NF:(rb + 1) * NF]
                            else:
                                y_stage = work.tile(
                                    [P, NF], F32, tag="ystage",
                                    bufs=int(os.environ.get("K_YSBUFS", "6")),
                                    name=f"yst_{rep}_{img}_{m}_{rb}")[:]
                            nc.scalar.activation(
                                y_stage, ps[:], AF.Copy,
                                accum_out=sum_cols[m][:, idx:idx + 1])
                            if convonly:
                                nc.sync.dma_start(
                                    o_flat[img, m * P:(m + 1) * P,
                                           rb * NF:(rb + 1) * NF],
                                    y_stage)
                                continue
                            # sum of squares on ACT (NOTE: tensor_tensor_reduce
                            # hangs real HW through this runtime — do not use)
                            sq = work.tile(
                                [P, NF], F32, tag="sq",
                                bufs=int(os.environ.get("K_SQBUFS", "2")),
                                name=f"sq_{rep}_{img}_{m}_{rb}")
                            nc.scalar.activation(
                                sq[:], y_stage, AF.Square,
                                accum_out=sq_cols[m][:, idx:idx + 1])
                            if y_dram is not None:
                                nc.sync.dma_start(
                                    y_dram[img * 2 + m][:,
                                                        rb * NF:(rb + 1) * NF],
                                    y_stage)

                if convonly:
                    continue

                # ---------- global stats (AllReduce) ----------
                for m in range(2):
                    nc.vector.reduce_sum(stats_sb[:, 2 * m:2 * m + 1],
                                         sum_cols[m][:],
                                         axis=mybir.AxisListType.X)
                    nc.vector.reduce_sum(stats_sb[:, 2 * m + 1:2 * m + 2],
                                         sq_cols[m][:],
                                         axis=mybir.AxisListType.X)
                nc.sync.dma_start(in_b[:], stats_sb[:])
                if "nocoll" in variant:
                    nc.sync.dma_start(out_b[:], in_b[:])
                else:
                    if "nocc" in variant:
                        groups = [[i] for i in range(n_cores)]
                    else:
                        groups = [list(range(n_cores))]
                    nc.gpsimd.collective_compute(
                        "AllReduce", ALU.add,
                        replica_groups=groups,
                        ins=[in_b.opt()], outs=[out_b.opt()],
                    )
                nc.sync.dma_start(gstats[:], out_b[:])

                # mean = S1/CNT; var = S2/CNT - mean^2
                # scale = gamma*rsqrt(var+eps); shift = beta - mean*scale
                for m in range(2):
                    nc.vector.tensor_scalar_mul(mean_sb[:, m:m + 1],
                                                gstats[:, 2 * m:2 * m + 1],
                                                1.0 / CNT)
                    nc.vector.tensor_scalar_mul(var_sb[:, m:m + 1],
                                                gstats[:, 2 * m + 1:2 * m + 2],
                                                1.0 / CNT)
                nc.vector.tensor_mul(msq[:], mean_sb[:], mean_sb[:])
                nc.vector.tensor_sub(var_sb[:], var_sb[:], msq[:])
                nc.vector.tensor_scalar_add(var_sb[:], var_sb[:], EPS)
                nc.vector.reciprocal(inv_sb[:], var_sb[:])
                nc.scalar.activation(rstd_sb[:], inv_sb[:], AF.Sqrt)
                nc.vector.tensor_mul(scale_sb[:], rstd_sb[:], gamma_sb[:])
                nc.vector.tensor_mul(mscale[:], mean_sb[:], scale_sb[:])
                nc.vector.tensor_sub(shift_sb[:], beta_sb[:], mscale[:])

                # ---------- apply: out = sign(y*scale + shift) ----------
                if "applyfine" in variant:
                    # finer tiles -> deeper DMA/ACT/DMA pipelining in the tail
                    for img in range(NIMG):
                        for m in range(2):
                            for rb in range(NRB):
                                sl = slice(rb * NF, (rb + 1) * NF)
                                y_ap = work.tile(
                                    [P, NF], F32, tag="yap", bufs=8,
                                    name=f"yap_{rep}_{img}_{m}_{rb}")
                                nc.sync.dma_start(
                                    y_ap[:], y_dram[img * 2 + m][:, sl])
                                o_sb = work.tile(
                                    [P, NF], F32, tag="osb", bufs=8,
                                    name=f"osb_{rep}_{img}_{m}_{rb}")
                                nc.scalar.activation(
                                    o_sb[:], y_ap[:], AF.Sign,
                                    bias=shift_sb[:, m:m + 1],
                                    scale=scale_sb[:, m:m + 1])
                                nc.sync.dma_start(
                                    o_flat[img, m * P:(m + 1) * P, sl],
                                    o_sb[:])
                elif y_sb is not None:
                    for img in range(NIMG):
                        for m in range(2):
                            o_sb = work.tile([P, H * W], F32, tag="osb",
                                             bufs=2,
                                             name=f"osb_{rep}_{img}_{m}")
                            nc.scalar.activation(
                                o_sb[:], y_sb[img * 2 + m][:], AF.Sign,
                                bias=shift_sb[:, m:m + 1],
                                scale=scale_sb[:, m:m + 1])
                            nc.sync.dma_start(
                                o_flat[img, m * P:(m + 1) * P, :], o_sb[:])
                else:
                    for img in range(NIMG):
                        for m in range(2):
                            y_ap = work.tile([P, H * W], F32, tag="yap",
                                             bufs=3,
                                             name=f"yap_{rep}_{img}_{m}")
                            nc.sync.dma_start(y_ap[:],
                                              y_dram[img * 2 + m][:, :])
                            o_sb = work.tile([P, H * W], F32, tag="osb",
                                             bufs=3,
                                             name=f"osb_{rep}_{img}_{m}")
                            nc.scalar.activation(
                                o_sb[:], y_ap[:], AF.Sign,
                                bias=shift_sb[:, m:m + 1],
                                scale=scale_sb[:, m:m + 1])
                            nc.sync.dma_start(
                                o_flat[img, m * P:(m + 1) * P, :], o_sb[:])

    nc.compile()
    return nc


def _get_nc(repeat=1):
    key = ("nc", repeat)
    if key not in _CACHE:
        _CACHE[key] = _build(repeat=repeat)
    return _CACHE[key]


def _prep_inputs(x, weight, gamma, beta):
    x = np.asarray(x, dtype=np.float32)
    weight = np.asarray(weight, dtype=np.float32)
    gamma = np.asarray(gamma, dtype=np.float32)
    beta = np.asarray(beta, dtype=np.float32)
    # sign-binarize + transpose to [kh*3+kw, ci, co]; +-1 is exact in
    # every matmul dtype (bf16 and fp32r alike)
    wt = np.sign(weight).transpose(2, 3, 1, 0).reshape(9, CI, CO)
    if "bf16" in os.environ.get("KERNEL_VARIANT", ""):
        wt = np.ascontiguousarray(wt).astype(ml_dtypes.bfloat16)
    else:
        wt = np.ascontiguousarray(wt, dtype=np.float32)
    in_maps = []
    for c in range(N_CORES):
        in_maps.append({
            "x": np.ascontiguousarray(x[c * NIMG:(c + 1) * NIMG]),
            "wt": wt,
            "gamma": gamma,
            "beta": beta,
        })
    return in_maps


def _run(inputs, trace=False, repeat=1):
    nc = _get_nc(repeat=repeat)
    in_maps = _prep_inputs(**inputs)
    res = bass_utils.run_bass_kernel_spmd(
        nc, in_maps, core_ids=list(range(N_CORES)), trace=trace)
    out = np.concatenate([res.results[c]["out"] for c in range(N_CORES)],
                         axis=0)
    return out, res


def kernel(**inputs):
    out, _ = _run(inputs, trace=False)
    return out
